# revision 7
# baseline (speedup 1.0000x reference)
"""Trainium2 Bass kernel for the RNN-T JointNetwork problem.

  enc = h_enc @ W_enc + b_enc            (B,T,1,J)
  dec = h_dec @ W_dec                    (B,1,U,J)
  z   = tanh(enc + dec)                  (B,T,U,J)
  out = z @ W_out + b_out                (B,T,U,V)

Shapes: B=4, T=256, U=64, D=J=V=512, fp32 in/out.

Sharding: 8 cores, data parallel over (B x T/2): core c handles batch
b = c//2 and t-half th = c%2 (128 t values). Params replicated.

v3 design (measured v2 = 99.5us, PE-bound incl. 10us of bias matmuls):
  - TRANSPOSED OUTPUT: the final matmul computes outT[v, row] with
    W_out chunks as the stationary operand and zT as the moving one.
    b_out becomes a per-partition scalar, so PSUM evacuation + bias
    runs as DVE tensor_scalar_add / ACT Identity-with-bias (split
    50/50) and the PE does exactly 256 main matmuls, nothing else.
    The host un-transposes during the gather (~0.26s for all cores).
  - zpre = encT (+) decT broadcast-add in fp32 (measured faster than
    bf16 at 1x: 1132 vs 1501 ns) on DVE; jc=0 adds on GpSimd.
  - One batched tanh per group ([128, 4096], amortizes the 352-cycle
    ACT bubble), bf16 output.
  - All matmul operands bf16 (host pre-cast), packed one-DMA-each.
  - Output stored/DMA'd bf16, host upcasts.
"""

import numpy as np

B, T, U = 4, 256, 64
D, J, V = 512, 512, 512
NCORES = 8
TH = T // 2          # t's per core = 128
KC = 4               # 512/128 contraction chunks
TG = 16              # t's per group
NG = TH // TG        # 8 groups
RB_PER_G = TG * U // 512   # 2 row-blocks (512 rows) per group
VQ = 4               # v-quarters (output partition chunks)

# ---- tuning knobs ----
GP_JC = {0}          # zpre adds for these jc run on GpSimd instead of DVE
Z_FP32 = True        # zpre tile dtype fp32 (False -> bf16)
# variable t-group sizes: small first group shortens the pipeline fill
# (first tanh comes sooner), small last group shortens the drain
GROUP_T = [8] + [16] * 7 + [8]
assert sum(GROUP_T) == TH

_compiled = None


def _build():
    import concourse.bass as bass
    import concourse.tile as tile
    from concourse import mybir

    fp32 = mybir.dt.float32
    bf16 = mybir.dt.bfloat16
    AF = mybir.ActivationFunctionType
    zdt = fp32 if Z_FP32 else bf16

    nc = bass.Bass()

    # chunk-interleaved packed layouts, one DMA each (see host packing)
    henct = nc.declare_dram_parameter("henct", [128, KC * TH], bf16, isOutput=False)
    hdect = nc.declare_dram_parameter("hdect", [128, KC * U], bf16, isOutput=False)
    wenc = nc.declare_dram_parameter("wenc", [128, KC * J], bf16, isOutput=False)
    wdec = nc.declare_dram_parameter("wdec", [128, KC * J], bf16, isOutput=False)
    # woutT[p, (jc*VQ + vq)*128 + m] = W_out[jc*128 + p, vq*128 + m]
    wout = nc.declare_dram_parameter("wout", [128, KC * V], bf16, isOutput=False)
    benc = nc.declare_dram_parameter("benc", [128, KC], fp32, isOutput=False)
    boutp = nc.declare_dram_parameter("boutp", [128, VQ], fp32, isOutput=False)
    out = nc.declare_dram_parameter("out", [V, TH * U], bf16, isOutput=True)

    with tile.TileContext(nc) as tc:
        with (
            tc.tile_pool(name="const", bufs=1) as const,
            tc.tile_pool(name="zpre", bufs=2) as zpre_pool,
            tc.tile_pool(name="zt", bufs=2) as zt_pool,
            tc.tile_pool(name="outs", bufs=6) as outs_pool,
            tc.tile_pool(name="ps_setup", bufs=1, space="PSUM") as ps_setup,
            tc.tile_pool(name="ps_out", bufs=6, space="PSUM") as ps_out,
        ):
            # ---- ACT table warmup: force the Tanh ACT_TABLE_LOAD to run
            # during the input-DMA window instead of on the critical path
            # right before the first real tanh (pool tiles are zero-inited
            # by the framework preamble, so this computes tanh(0)=0).
            warm = const.tile([1, 2], fp32, tag="warm")
            nc.scalar.activation(warm[0:1, 1:2], warm[0:1, 0:1], AF.Tanh)

            # ---- load everything to SBUF (one DMA per tensor) ----
            henct_s = const.tile([128, KC * TH], bf16, tag="henct")
            nc.sync.dma_start(henct_s[:], henct[:])
            wenc_s = const.tile([128, KC * J], bf16, tag="wenc")
            nc.sync.dma_start(wenc_s[:], wenc[:])
            benc_s = const.tile([128, KC], fp32, tag="benc")
            nc.sync.dma_start(benc_s[:], benc[:])
            hdect_s = const.tile([128, KC * U], bf16, tag="hdect")
            nc.sync.dma_start(hdect_s[:], hdect[:])
            wdec_s = const.tile([128, KC * J], bf16, tag="wdec")
            nc.sync.dma_start(wdec_s[:], wdec[:])
            wout_s = const.tile([128, KC * V], bf16, tag="wout")
            nc.sync.dma_start(wout_s[:], wout[:])
            boutp_s = const.tile([128, VQ], fp32, tag="boutp")
            nc.sync.dma_start(boutp_s[:], boutp[:])

            # ---- encT / decT (bf16 setup matmuls; evacuations on ACT to
            # keep the in-order DVE stream free for the first zpre adds) ----
            encT_s = []
            decT_s = []
            for jc in range(KC):
                pe = ps_setup.tile([128, TH], fp32, tag="pse")
                for k in range(KC):
                    nc.tensor.matmul(
                        pe[:],
                        wenc_s[:, k * J + jc * 128:k * J + (jc + 1) * 128],
                        henct_s[:, k * TH:(k + 1) * TH],
                        start=(k == 0),
                        stop=(k == KC - 1),
                    )
                et = const.tile([128, TH], zdt, tag=f"encT{jc}")
                # encT = psum + b_enc[jc] (per-partition bias on ACT)
                nc.scalar.activation(
                    et[:], pe[:], AF.Identity, bias=benc_s[:, jc:jc + 1]
                )
                encT_s.append(et)

                pd = ps_setup.tile([128, U], fp32, tag="psd")
                for k in range(KC):
                    nc.tensor.matmul(
                        pd[:],
                        wdec_s[:, k * J + jc * 128:k * J + (jc + 1) * 128],
                        hdect_s[:, k * U:(k + 1) * U],
                        start=(k == 0),
                        stop=(k == KC - 1),
                    )
                dt_ = const.tile([128, U], zdt, tag=f"decT{jc}")
                nc.scalar.activation(dt_[:], pd[:], AF.Copy)
                decT_s.append(dt_)

            # ---- main loop ----
            evac_ctr = 0
            col0 = 0
            for g, tg in enumerate(GROUP_T):
                gw = tg * U          # columns of z this group covers
                nrb = gw // 512      # 512-row blocks
                zp = zpre_pool.tile([128, KC * 16 * U], zdt, tag="zp")
                for jc in range(KC):
                    zps = zp[:, jc * gw:(jc + 1) * gw]
                    zp3 = zps.rearrange("p (t u) -> p t u", t=tg)
                    d3 = (
                        decT_s[jc][:]
                        .rearrange("p (x u) -> p x u", x=1)
                        .to_broadcast([128, tg, U])
                    )
                    e3 = (
                        encT_s[jc][:, col0 // U:col0 // U + tg]
                        .rearrange("p (t x) -> p t x", x=1)
                        .to_broadcast([128, tg, U])
                    )
                    eng = nc.gpsimd if jc in GP_JC else nc.vector
                    eng.tensor_add(zp3, d3, e3)
                zt = zt_pool.tile([128, KC * 16 * U], bf16, tag="zt")
                nc.scalar.activation(zt[:, :KC * gw], zp[:, :KC * gw], AF.Tanh)

                for vq in range(VQ):
                    for rb in range(nrb):
                        po = ps_out.tile([128, 512], fp32, tag="po")
                        for jc in range(KC):
                            nc.tensor.matmul(
                                po[:],
                                wout_s[:, (jc * VQ + vq) * 128:(jc * VQ + vq + 1) * 128],
                                zt[:, jc * gw + rb * 512:jc * gw + rb * 512 + 512],
                                start=(jc == 0),
                                stop=(jc == KC - 1),
                            )
                        ob = outs_pool.tile([128, 512], bf16, tag="ob")
                        if evac_ctr % 2 == 1:
                            nc.scalar.activation(
                                ob[:], po[:], AF.Identity,
                                bias=boutp_s[:, vq:vq + 1],
                            )
                        else:
                            nc.vector.tensor_scalar_add(
                                ob[:], po[:], boutp_s[:, vq:vq + 1]
                            )
                        evac_ctr += 1
                        c0 = col0 + rb * 512
                        nc.sync.dma_start(
                            out[vq * 128:(vq + 1) * 128, c0:c0 + 512],
                            ob[:],
                        )
                col0 += gw

    _split_multi_waits(nc)
    return nc


_COMPUTE_OPS = {
    "Matmult", "Ldweights", "TensorTensor", "TensorCopy", "TensorScalarPtr",
    "Activation", "TensorReduce", "Memset", "ScalarTensorTensor",
    "TensorScalar", "DMACopy", "Drain", "EventSemaphore",
}


def _split_multi_waits(nc):
    """walrus codegen in this container allows a single sync-wait command
    per TPB compute instruction; Tile emits several.  Hoist all but one
    wait onto standalone EventSemaphore instructions placed just before
    the offending instruction (same engine, so semantics are identical).
    """
    from concourse import mybir

    ctr = [0]
    for fn in nc.m.functions:
        for blk in fn.blocks:
            insts = blk.instructions
            out = []
            for inst in insts:
                si = getattr(inst, "sync_info", None)
                ow = list(si.on_wait) if si and si.on_wait else []
                if (
                    len(ow) > 1
                    and getattr(inst, "opcode", None) in _COMPUTE_OPS
                ):
                    for w in ow[:-1]:
                        ctr[0] += 1
                        ev = mybir.InstEventSemaphore(
                            name=f"WS-{ctr[0]}-{inst.name}",
                            ins=[],
                            outs=[],
                            sync_info=mybir.SyncInfo(
                                on_wait=[w], on_update=[]
                            ),
                        )
                        ev.engine = inst.engine
                        out.append(ev)
                    inst.sync_info = mybir.SyncInfo(
                        on_wait=[ow[-1]], on_update=list(si.on_update or [])
                    )
                out.append(inst)
            blk.instructions = out


def _get_compiled():
    global _compiled
    if _compiled is None:
        _compiled = _build()
    return _compiled


def _pack_chunks(mat, ncols):
    """[D, N] (contraction-major) -> [128, KC*N] bf16, chunk-interleaved:
    out[p, k*N + n] = mat[k*128 + p, n]"""
    import ml_dtypes

    m = np.asarray(mat, dtype=np.float32).reshape(KC, 128, ncols)
    m = m.transpose(1, 0, 2).reshape(128, KC * ncols)
    return np.ascontiguousarray(m.astype(ml_dtypes.bfloat16))


def kernel(h_enc, h_dec, W_enc, b_enc, W_dec, W_out, b_out, **_):
    import ml_dtypes

    nc = _get_compiled()
    from concourse.bass_utils import run_bass_kernel_spmd

    h_enc = np.asarray(h_enc, dtype=np.float32)
    h_dec = np.asarray(h_dec, dtype=np.float32)

    wenc_p = _pack_chunks(W_enc, J)
    wdec_p = _pack_chunks(W_dec, J)
    # stationary W_out chunks: [p, (jc*VQ+vq)*128 + m] = W_out[jc*128+p, vq*128+m]
    wout_p = np.ascontiguousarray(
        np.asarray(W_out, dtype=np.float32)
        .reshape(KC, 128, VQ, 128)
        .transpose(1, 0, 2, 3)
        .reshape(128, KC * V)
        .astype(ml_dtypes.bfloat16)
    )
    benc_cols = np.ascontiguousarray(
        np.asarray(b_enc, dtype=np.float32).reshape(KC, 128).T
    )
    boutp = np.ascontiguousarray(
        np.asarray(b_out, dtype=np.float32).reshape(VQ, 128).T
    )

    in_maps = []
    for c in range(NCORES):
        b, th = c // 2, c % 2
        henct_p = _pack_chunks(h_enc[b, th * TH:(th + 1) * TH, 0, :].T, TH)
        hdect_p = _pack_chunks(h_dec[b, 0, :, :].T, U)
        in_maps.append(
            {
                "henct": henct_p,
                "hdect": hdect_p,
                "wenc": wenc_p,
                "wdec": wdec_p,
                "wout": wout_p,
                "benc": benc_cols,
                "boutp": boutp,
            }
        )

    global _last_in_maps
    _last_in_maps = in_maps
    res = run_bass_kernel_spmd(nc, in_maps, list(range(NCORES)))

    out_full = np.empty((B, T, U, V), dtype=np.float32)
    for c in range(NCORES):
        b, th = c // 2, c % 2
        outT = np.asarray(res.results[c]["out"])  # [V, TH*U] bf16
        out_full[b, th * TH:(th + 1) * TH] = (
            outT.astype(np.float32).T.reshape(TH, U, V)
        )
    return out_full


# revision 8
# speedup vs baseline: 1.0834x; 1.0834x over previous
"""Trainium2 Bass kernel for the RNN-T JointNetwork problem.

  enc = h_enc @ W_enc + b_enc            (B,T,1,J)
  dec = h_dec @ W_dec                    (B,1,U,J)
  z   = tanh(enc + dec)                  (B,T,U,J)
  out = z @ W_out + b_out                (B,T,U,V)

Shapes: B=4, T=256, U=64, D=J=V=512, fp32 in/out.

Sharding: 8 cores, data parallel over (B x T/2): core c handles batch
b = c//2 and t-half th = c%2 (128 t values). Params replicated.

v5 design (v3=96.5us, v4=106us):
  - TRANSPOSED OUTPUT (outT[v, row]): W_out chunks stationary, zT
    moving, b_out per-partition -> evacuation via DVE tensor_scalar_add
    / ACT Identity-with-bias, PE does only the 256 main matmuls.
  - U-MAJOR z rows within each t-group: row = u*tg + t_local.  Both
    zpre-add operands are then innermost-step-1 APs:
      dec side: dec_repU[j, u, t] = decT[j, u] (one-time GpSimd build)
      enc side: encT[j, t-slice] broadcast over the OUTER u dim.
    This enables the DVE 2x_1P mode for bf16 adds (594ns vs 1133ns
    per [128,1024] chunk).  The host un-permutes columns on gather.
  - First/last groups are small (8 t's) to shorten pipeline fill and
    drain; the first two groups use direct decT-broadcast adds so they
    don't wait on the GpSimd dec_repU build.
  - 3 input DMAs total (packed blobs) - the serial ~650ns/DMA
    DIRECT2D descriptor-gen on Sync was stretching the input phase.
  - ACT table preloaded with a dummy tanh at t~0.
"""

import numpy as np

B, T, U = 4, 256, 64
D, J, V = 512, 512, 512
NCORES = 8
TH = T // 2          # t's per core = 128
KC = 4               # 512/128 contraction chunks
VQ = 4               # v-quarters (output partition chunks)
MAXTG = 16

# ---- tuning knobs ----
Z_FP32 = False       # zpre dtype fp32 (True) or bf16 (False; enables 2x adds)
GP_JC = set()        # zpre adds for these jc run on GpSimd instead of DVE
GROUP_T = [8] + [16] * 7 + [8]
assert sum(GROUP_T) == TH
N_DIRECT = 2         # first N groups use direct decT-broadcast adds

_compiled = None

# blob1: henct | wenc   (bf16)
B1_HENC, B1_WENC, B1_COLS = 0, KC * TH, KC * TH + KC * J
# blob2: hdect | wdec | wout  (bf16)
B2_HDEC, B2_WDEC, B2_WOUT = 0, KC * U, KC * U + KC * J
B2_COLS = KC * U + 2 * KC * J
# blob3: benc | boutp  (fp32)
B3_COLS = 2 * KC


def _build():
    import concourse.bass as bass
    import concourse.tile as tile
    from concourse import mybir

    fp32 = mybir.dt.float32
    bf16 = mybir.dt.bfloat16
    AF = mybir.ActivationFunctionType
    zdt = fp32 if Z_FP32 else bf16

    nc = bass.Bass()

    blob1 = nc.declare_dram_parameter("blob1", [128, B1_COLS], bf16, isOutput=False)
    blob2 = nc.declare_dram_parameter("blob2", [128, B2_COLS], bf16, isOutput=False)
    blob3 = nc.declare_dram_parameter("blob3", [128, B3_COLS], fp32, isOutput=False)
    out = nc.declare_dram_parameter("out", [V, TH * U], bf16, isOutput=True)

    with tile.TileContext(nc) as tc:
        with (
            tc.tile_pool(name="const", bufs=1) as const,
            tc.tile_pool(name="zpre", bufs=3) as zpre_pool,
            tc.tile_pool(name="zt", bufs=3) as zt_pool,
            tc.tile_pool(name="outs", bufs=6) as outs_pool,
            tc.tile_pool(name="ps_setup", bufs=2, space="PSUM") as ps_setup,
            tc.tile_pool(name="ps_out", bufs=6, space="PSUM") as ps_out,
        ):
            # ---- ACT table warmup (pool tiles are zero-inited) ----
            warm = const.tile([1, 2], fp32, tag="warm")
            nc.scalar.activation(warm[0:1, 1:2], warm[0:1, 0:1], AF.Tanh)

            # ---- load everything to SBUF (3 packed DMAs) ----
            b1 = const.tile([128, B1_COLS], bf16, tag="b1")
            nc.sync.dma_start(b1[:], blob1[:])
            b2 = const.tile([128, B2_COLS], bf16, tag="b2")
            nc.sync.dma_start(b2[:], blob2[:])
            b3 = const.tile([128, B3_COLS], fp32, tag="b3")
            nc.sync.dma_start(b3[:], blob3[:])

            def henct_k(k):
                return b1[:, B1_HENC + k * TH:B1_HENC + (k + 1) * TH]

            def wenc_kj(k, jc):
                c = B1_WENC + k * J + jc * 128
                return b1[:, c:c + 128]

            def hdect_k(k):
                return b2[:, B2_HDEC + k * U:B2_HDEC + (k + 1) * U]

            def wdec_kj(k, jc):
                c = B2_WDEC + k * J + jc * 128
                return b2[:, c:c + 128]

            def wout_jv(jc, vq):
                c = B2_WOUT + (jc * VQ + vq) * 128
                return b2[:, c:c + 128]

            benc_s = b3[:, 0:KC]
            boutp_s = b3[:, KC:2 * KC]

            # ---- encT / decT (bf16 setup matmuls; evacs on ACT) ----
            encT_s = []
            decT_s = []
            for jc in range(KC):
                pe = ps_setup.tile([128, TH], fp32, tag="ps")
                for k in range(KC):
                    nc.tensor.matmul(
                        pe[:],
                        wenc_kj(k, jc),
                        henct_k(k),
                        start=(k == 0),
                        stop=(k == KC - 1),
                    )
                et = const.tile([128, TH], zdt, tag=f"encT{jc}")
                nc.scalar.activation(
                    et[:], pe[:], AF.Identity, bias=benc_s[:, jc:jc + 1]
                )
                encT_s.append(et)

                pd = ps_setup.tile([128, TH], fp32, tag="ps")
                for k in range(KC):
                    nc.tensor.matmul(
                        pd[:, 0:U],
                        wdec_kj(k, jc),
                        hdect_k(k),
                        start=(k == 0),
                        stop=(k == KC - 1),
                    )
                dt_ = const.tile([128, U], zdt, tag=f"decT{jc}")
                nc.scalar.activation(dt_[:], pd[:, 0:U], AF.Copy)
                decT_s.append(dt_)

            # ---- dec_repU[jc][j, u, t] = decT[jc][j, u]  (GpSimd, one-time;
            # groups 0..N_DIRECT-1 don't wait for it) ----
            dec_repU = []
            for jc in range(KC):
                dr = const.tile([128, U * MAXTG], zdt, tag=f"drepU{jc}")
                dr3 = dr[:].rearrange("p (u t) -> p u t", u=U)
                nc.gpsimd.tensor_copy(
                    dr3,
                    decT_s[jc][:]
                    .rearrange("p (u x) -> p u x", x=1)
                    .to_broadcast([128, U, MAXTG]),
                )
                dec_repU.append(dr)

            # ---- main loop ----
            evac_ctr = 0
            col0 = 0
            for g, tg in enumerate(GROUP_T):
                gw = tg * U          # z columns this group covers
                nrb = gw // 512
                zp = zpre_pool.tile([128, KC * MAXTG * U], zdt, tag="zp")
                for jc in range(KC):
                    zps = zp[:, jc * gw:(jc + 1) * gw]
                    zp3 = zps.rearrange("p (u t) -> p u t", u=U)
                    if g < N_DIRECT:
                        d3 = (
                            decT_s[jc][:]
                            .rearrange("p (u x) -> p u x", x=1)
                            .to_broadcast([128, U, tg])
                        )
                    else:
                        d3 = (
                            dec_repU[jc][:]
                            .rearrange("p (u t) -> p u t", u=U)[:, :, 0:tg]
                        )
                    e3 = (
                        encT_s[jc][:, col0 // U:col0 // U + tg]
                        .rearrange("p (x t) -> p x t", x=1)
                        .to_broadcast([128, U, tg])
                    )
                    eng = nc.gpsimd if jc in GP_JC else nc.vector
                    eng.tensor_add(zp3, d3, e3)
                zt = zt_pool.tile([128, KC * MAXTG * U], bf16, tag="zt")
                nc.scalar.activation(zt[:, :KC * gw], zp[:, :KC * gw], AF.Tanh)

                for vq in range(VQ):
                    for rb in range(nrb):
                        po = ps_out.tile([128, 512], fp32, tag="po")
                        for jc in range(KC):
                            nc.tensor.matmul(
                                po[:],
                                wout_jv(jc, vq),
                                zt[:, jc * gw + rb * 512:jc * gw + rb * 512 + 512],
                                start=(jc == 0),
                                stop=(jc == KC - 1),
                            )
                        ob = outs_pool.tile([128, 512], bf16, tag="ob")
                        if evac_ctr % 2 == 1:
                            nc.scalar.activation(
                                ob[:], po[:], AF.Identity,
                                bias=boutp_s[:, vq:vq + 1],
                            )
                        else:
                            nc.vector.tensor_scalar_add(
                                ob[:], po[:], boutp_s[:, vq:vq + 1]
                            )
                        evac_ctr += 1
                        c0 = col0 + rb * 512
                        nc.sync.dma_start(
                            out[vq * 128:(vq + 1) * 128, c0:c0 + 512],
                            ob[:],
                        )
                col0 += gw

    _split_multi_waits(nc)
    return nc


_COMPUTE_OPS = {
    "Matmult", "Ldweights", "TensorTensor", "TensorCopy", "TensorScalarPtr",
    "Activation", "TensorReduce", "Memset", "ScalarTensorTensor",
    "TensorScalar", "DMACopy", "Drain", "EventSemaphore",
}


def _split_multi_waits(nc):
    """walrus codegen in this container allows a single sync-wait command
    per TPB compute instruction; Tile emits several.  Hoist all but one
    wait onto standalone EventSemaphore instructions placed just before
    the offending instruction (same engine, so semantics are identical).
    """
    from concourse import mybir

    ctr = [0]
    for fn in nc.m.functions:
        for blk in fn.blocks:
            insts = blk.instructions
            out = []
            for inst in insts:
                si = getattr(inst, "sync_info", None)
                ow = list(si.on_wait) if si and si.on_wait else []
                if (
                    len(ow) > 1
                    and getattr(inst, "opcode", None) in _COMPUTE_OPS
                ):
                    for w in ow[:-1]:
                        ctr[0] += 1
                        ev = mybir.InstEventSemaphore(
                            name=f"WS-{ctr[0]}-{inst.name}",
                            ins=[],
                            outs=[],
                            sync_info=mybir.SyncInfo(
                                on_wait=[w], on_update=[]
                            ),
                        )
                        ev.engine = inst.engine
                        out.append(ev)
                    inst.sync_info = mybir.SyncInfo(
                        on_wait=[ow[-1]], on_update=list(si.on_update or [])
                    )
                out.append(inst)
            blk.instructions = out


def _get_compiled():
    global _compiled
    if _compiled is None:
        _compiled = _build()
    return _compiled


def _chunk_rows(mat, ncols):
    """[D, N] (contraction-major) -> [128, KC*N] f32, chunk-interleaved:
    out[p, k*N + n] = mat[k*128 + p, n]"""
    m = np.asarray(mat, dtype=np.float32).reshape(KC, 128, ncols)
    return m.transpose(1, 0, 2).reshape(128, KC * ncols)


def kernel(h_enc, h_dec, W_enc, b_enc, W_dec, W_out, b_out, **_):
    import ml_dtypes

    nc = _get_compiled()
    from concourse.bass_utils import run_bass_kernel_spmd

    bf16 = ml_dtypes.bfloat16
    h_enc = np.asarray(h_enc, dtype=np.float32)
    h_dec = np.asarray(h_dec, dtype=np.float32)

    wenc_p = _chunk_rows(W_enc, J)
    wdec_p = _chunk_rows(W_dec, J)
    wout_p = (
        np.asarray(W_out, dtype=np.float32)
        .reshape(KC, 128, VQ, 128)
        .transpose(1, 0, 2, 3)
        .reshape(128, KC * V)
    )
    blob3 = np.ascontiguousarray(
        np.concatenate(
            [
                np.asarray(b_enc, dtype=np.float32).reshape(KC, 128).T,
                np.asarray(b_out, dtype=np.float32).reshape(VQ, 128).T,
            ],
            axis=1,
        )
    )

    hdec_b = {}
    in_maps = []
    for c in range(NCORES):
        b, th = c // 2, c % 2
        henct_p = _chunk_rows(h_enc[b, th * TH:(th + 1) * TH, 0, :].T, TH)
        blob1 = np.ascontiguousarray(
            np.concatenate([henct_p, wenc_p], axis=1).astype(bf16)
        )
        if b not in hdec_b:
            hdect_p = _chunk_rows(h_dec[b, 0, :, :].T, U)
            hdec_b[b] = np.ascontiguousarray(
                np.concatenate([hdect_p, wdec_p, wout_p], axis=1).astype(bf16)
            )
        in_maps.append({"blob1": blob1, "blob2": hdec_b[b], "blob3": blob3})

    global _last_in_maps
    _last_in_maps = in_maps
    res = run_bass_kernel_spmd(nc, in_maps, list(range(NCORES)))

    out_full = np.empty((B, T, U, V), dtype=np.float32)
    for c in range(NCORES):
        b, th = c // 2, c % 2
        outT = np.asarray(res.results[c]["out"]).astype(np.float32)  # [V, 8192]
        t0 = th * TH
        col0 = 0
        for tg in GROUP_T:
            gw = tg * U
            blk = outT[:, col0:col0 + gw].reshape(V, U, tg)
            # [v, u, t] -> [t, u, v]
            out_full[b, t0:t0 + tg] = blk.transpose(2, 1, 0)
            t0 += tg
            col0 += gw
    return out_full
